# revision 6
# baseline (speedup 1.0000x reference)
"""Trainium2 Bass kernel for nn_BlockSelfAttention (attention over 8 heads per token).

Math per token t: xh = x[t].reshape(8,128); q/k/v = xh@W.T+b;
out[t] = softmax(q k^T/sqrt(128)) @ v.  Identities: bk drops out of softmax;
1/sqrt(d) and bq fold into zmt/ucol (z-trick: scores = x . (s Wq^T Wk) x, one
projection instead of two); bv added to V rows.

Schedule (per core: 4096 tokens = 32 tiles of 128; TimelineSim 99.6us):
  * engines: ALL PSUM<->SBUF elementwise traffic must go through ACT/DVE
    (GPSIMD/Pool and DMA cannot touch PSUM - walrus verifier).
    ACT: z-copy+ucol bias (both halves), exp (both halves), denominator
    stage copy.  DVE: V+bv (2), out*recip (2), reciprocal (from SBUF).
    DVE is the bound: 4x658+69 = 2701 ns/tile, fully saturated.
  * PSUM rings sized so no next-tile PE op waits on exp:
    ps ring 4 bufs (zpsA zpsB spsA spsB + dps), psv 2 (vpsA vpsB),
    pav 2 (avA avB) = 8 banks.
  * one-stage software pipeline: AV + out-normalize of tile i issue after
    tile i+1's front; denominators+reciprocal stay in the front so rsb is
    long-ready before the out-muls.
  * bf16 output DMA (host converts to f32; rel_err ~4e-3 vs 2e-2 gate).
  * startup: zmt then XT(0) head the HWDGE queue; other consts go via
    gpsimd/SWDGE so the first z-matmul isn't DMA-queued behind them.
"""

import numpy as np

HEADS = 8
D = 128
B, N, F = 8, 4096, 1024
NCORES = 8
TOK = (B * N) // NCORES          # tokens per core
P = 128                          # tokens per tile
NT = TOK // P                    # tiles per core
NEG = -30000.0

_NC_CACHE = {}


def _build_nc(mm_dt_name="f32", BUFS=None, reps=1, FB_ORDER="fb"):
    import concourse.mybir as mybir
    import concourse.tile as tile
    from concourse import bacc
    from contextlib import ExitStack

    f32 = mybir.dt.float32
    bf16 = mybir.dt.bfloat16
    if mm_dt_name in ("f32", "f32r"):
        mm_dt = f32
    elif mm_dt_name == "bf16":
        mm_dt = bf16
    else:
        raise ValueError(mm_dt_name)

    def mm(ap):
        # reinterpret an fp32 AP as fp32r at matmul call sites
        if mm_dt_name == "f32r" and ap.dtype == f32:
            return ap.bitcast(mybir.dt.float32r)
        return ap

    BUFS = BUFS or {}
    SCR_OUT = bool(BUFS.get("scr_out", 1))
    nc = bacc.Bacc("TRN2", target_bir_lowering=False, debug=False)

    xt = nc.dram_tensor("xt", [D, TOK * HEADS], mm_dt, kind="ExternalInput")
    zmt = nc.dram_tensor("zmt", [D, D], mm_dt, kind="ExternalInput")
    wvt = nc.dram_tensor("wvt", [D, D], mm_dt, kind="ExternalInput")
    ucol = nc.dram_tensor("ucol", [D, 1], f32, kind="ExternalInput")
    bvb = nc.dram_tensor("bvb", [D, 512], f32, kind="ExternalInput")
    mka = nc.dram_tensor("mka", [32, D], bf16, kind="ExternalInput")
    mkb = nc.dram_tensor("mkb", [32, 4 * D], bf16, kind="ExternalInput")
    one = nc.dram_tensor("one", [D, 1], mm_dt, kind="ExternalInput")
    y = nc.dram_tensor("y", [TOK, F], bf16, kind="ExternalOutput")

    xt_r = xt.ap().rearrange("d (T c) -> T d c", c=P * HEADS)
    # DRAM element address for out tile T, group j, partition p=(t%16)*8+h, e:
    # (T*128 + 16j + p//8)*1024 + (p%8)*128 + e = T*131072 + j*16384 + p*128 + e
    if SCR_OUT:
        # scrambled: tile-row-major dump; host un-permutes
        y_r = y.ap().rearrange("(T p) c -> T p c", p=P)
    else:
        y_r = y.ap().flatten().rearrange(
            "(T j p e) -> T p j e", T=NT, j=8, p=P, e=D
        )

    AF = mybir.ActivationFunctionType

    with tile.TileContext(nc) as tc, ExitStack() as es:
        cpool = es.enter_context(tc.tile_pool(name="consts", bufs=1))
        zmt_s = cpool.tile([D, D], mm_dt, tag="zmt")
        wvt_s = cpool.tile([D, D], mm_dt, tag="wvt")
        ucol_s = cpool.tile([D, 1], f32, tag="ucol")
        bvb_s = cpool.tile([D, 512], f32, tag="bvb")
        mka_s = cpool.tile([32, D], bf16, tag="mka")
        mkb_s = cpool.tile([32, 4 * D], bf16, tag="mkb")
        one_s = cpool.tile([D, 1], mm_dt, tag="one")
        pxt = es.enter_context(tc.tile_pool(name="pxt", bufs=BUFS.get("pxt", 3)))
        pz = es.enter_context(tc.tile_pool(name="pz", bufs=BUFS.get("pz", 2)))
        pv = es.enter_context(tc.tile_pool(name="pv", bufs=BUFS.get("pv", 2)))
        ppt = es.enter_context(tc.tile_pool(name="ppt", bufs=BUFS.get("ppt", 3)))
        pdr = es.enter_context(tc.tile_pool(name="pdr", bufs=BUFS.get("pdr", 2)))
        po = es.enter_context(tc.tile_pool(name="po", bufs=BUFS.get("po", 3)))
        ps = es.enter_context(tc.tile_pool(
            name="ps", bufs=BUFS.get("ps", 4), space="PSUM"))
        psv = es.enter_context(tc.tile_pool(
            name="psv", bufs=BUFS.get("psv", 2), space="PSUM"))
        pav = es.enter_context(tc.tile_pool(
            name="pav", bufs=BUFS.get("pav", 2), space="PSUM"))

        nc.sync.dma_start(zmt_s[:], zmt.ap())
        nc.scalar.dma_start(ucol_s[:], ucol.ap())
        XT0 = pxt.tile([D, P * HEADS], mm_dt, tag="xt")
        nc.sync.dma_start(XT0[:], xt_r[0])
        for t_, d_ in (
            (mka_s, mka), (mkb_s, mkb), (wvt_s, wvt), (bvb_s, bvb),
            (one_s, one),
        ):
            nc.gpsimd.dma_start(t_[:], d_.ap())
        bvb_v = bvb_s[:].rearrange("p (j e) -> p j e", e=D)
        # warm the ACT exp table while the first DMAs are in flight
        warm = cpool.tile([1, 2], f32, tag="warm")
        nc.gpsimd.memset(warm[:], 0.0)
        nc.scalar.activation(warm[0:1, 0:1], warm[0:1, 1:2], AF.Exp)

        import contextlib
        rep_cm = tc.For_i(0, reps, 1) if reps > 1 else contextlib.nullcontext()
        def front(T):
              if T == 0:
                  XT = XT0
              else:
                  XT = pxt.tile([D, P * HEADS], mm_dt, tag="xt")
                  nc.sync.dma_start(XT[:], xt_r[T])

              # ---- z projection: zT2 = (s*Wk^T Wq) x + s*Wk^T bq ----
              # scoresT[(t,g),(t,h)] = x_g . z_h reproduces k.(q*s+bq*s)
              zT2 = pz.tile([D, P * HEADS], mm_dt, tag="z")
              for half in range(2):
                  csl = slice(512 * half, 512 * half + 512)
                  zps = ps.tile([D, 512], f32, tag="ps")
                  nc.tensor.matmul(zps[:], mm(zmt_s[:]), mm(XT[:, csl]),
                                   start=True, stop=True)
                  nc.scalar.activation(zT2[:, csl], zps[:], AF.Identity,
                                       bias=ucol_s[:, 0:1])

              # ---- v projection -> V [(t,g), j, e] ----
              V = pv.tile([P, 8, D], mm_dt, tag="v")
              for half in range(2):
                  vps = psv.tile([P, 4, D], f32, tag="vps")
                  for jj in range(4):
                      j = 4 * half + jj
                      nc.tensor.matmul(vps[:, jj, :],
                                       mm(XT[:, 128 * j:128 * j + 128]),
                                       mm(wvt_s[:]), start=True, stop=True)
                  nc.vector.tensor_add(V[:, 4 * half:4 * half + 4, :], vps[:],
                                       bvb_v[:])

              # ---- scores (transposed) + mask + exp -> PT [(t,g), j, (t,h)] ----
              PT = ppt.tile([P, 8, P], mm_dt, tag="pt")
              for half in range(2):
                  sps = ps.tile([P, 4, P], f32, tag="ps")
                  nc.tensor.matmul(sps[:], mka_s[:], mkb_s[:],
                                   start=True, stop=False)
                  for jj in range(4):
                      j = 4 * half + jj
                      gsl = slice(128 * j, 128 * j + 128)
                      nc.tensor.matmul(sps[:, jj, :], mm(XT[:, gsl]),
                                       mm(zT2[:, gsl]), start=False, stop=True,
                                       skip_group_check=True)
                  nc.scalar.activation(PT[:, 4 * half:4 * half + 4, :], sps[:],
                                       AF.Exp)

              dps = ps.tile([P, 8], f32, tag="ps")
              for j in range(8):
                  nc.tensor.matmul(dps[:, j:j + 1], mm(PT[:, j, :]),
                                   mm(one_s[:]), start=True, stop=True)
              dsb = pdr.tile([P, 8], f32, tag="ds")
              nc.scalar.activation(dsb[:], dps[:], AF.Identity)
              rsb = pdr.tile([P, 8], f32, tag="rs")
              nc.vector.reciprocal(rsb[:], dsb[:])
              return (PT, V, rsb, T)

        def back(st):
              PT, V, rsb, T = st
              avp = []
              for half in range(2):
                  avps = pav.tile([P, 4, D], f32, tag="av")
                  avp.append(avps)
                  for jj in range(4):
                      j = 4 * half + jj
                      nc.tensor.matmul(avps[:, jj, :], mm(PT[:, j, :]),
                                       mm(V[:, j, :]), start=True, stop=True)

              out = po.tile([P, 8, D], bf16, tag="o")
              for half in range(2):
                  hsl = slice(4 * half, 4 * half + 4)
                  nc.vector.tensor_mul(
                      out[:, hsl, :], avp[half][:],
                      rsb[:, hsl, None].broadcast_to([P, 4, D]))
              nc.sync.dma_start(
                  y_r[T], out[:].rearrange("p j e -> p (j e)"))

        with rep_cm:
          pend = None
          for T in range(NT):
              if FB_ORDER == "fb":
                  st = front(T)
                  if pend is not None:
                      back(pend)
              else:
                  st = None
                  if pend is not None:
                      back(pend)
                  st = front(T)
              pend = st
          back(pend)

    nc.compile()
    return nc


def _get_nc(mm_dt_name="f32"):
    if mm_dt_name not in _NC_CACHE:
        _NC_CACHE[mm_dt_name] = _build_nc(mm_dt_name)
    return _NC_CACHE[mm_dt_name]


def _prep_in_maps(x, Wq, bq, Wk, bk, Wv, bv, mm_dt_name="f32"):
    import ml_dtypes
    if mm_dt_name == "bf16":
        mm_np = ml_dtypes.bfloat16
    else:
        mm_np = np.float32
    s = np.float32(1.0 / np.sqrt(D))
    Wq = np.asarray(Wq, np.float64)
    Wk = np.asarray(Wk, np.float64)
    zmt = np.ascontiguousarray(s * (Wq.T @ Wk)).astype(mm_np)
    ucol = (s * (Wk.T @ np.asarray(bq, np.float64))).reshape(D, 1).astype(
        np.float32)
    wvt = np.ascontiguousarray(np.asarray(Wv).T).astype(mm_np)
    bvb = np.tile(np.asarray(bv).reshape(1, D).astype(np.float32), (D, 4))
    a = np.float32(np.sqrt(-NEG))
    mka = np.zeros((32, D), np.float32)
    mkb = np.zeros((32, D), np.float32)
    mka[0, :] = a
    mkb[0, :] = -a
    for j in range(16):
        mka[1 + j, 8 * j:8 * j + 8] = a
        mkb[1 + j, 8 * j:8 * j + 8] = a
    mka = mka.astype(ml_dtypes.bfloat16)
    mkb = np.tile(mkb, (1, 4)).astype(ml_dtypes.bfloat16)
    one = np.ones((D, 1), mm_np)
    xs = np.asarray(x, np.float32).reshape(B * N, F)
    shared = dict(zmt=zmt, wvt=wvt, ucol=ucol, bvb=bvb, mka=mka,
                  mkb=mkb, one=one)
    in_maps = []
    for c in range(NCORES):
        xc = xs[c * TOK:(c + 1) * TOK]
        # xt[d, t*8+h] = x[t, h*128+d]
        xtc = np.ascontiguousarray(
            xc.reshape(TOK, HEADS, D).transpose(2, 0, 1).reshape(
                D, TOK * HEADS)).astype(mm_np)
        in_maps.append(dict(xt=xtc, **shared))
    return in_maps


def run(x, Wq, bq, Wk, bk, Wv, bv, mm_dt_name="f32", run_bufs=None,
        **run_kw):
    from concourse.bass_utils import run_bass_kernel_spmd

    nc = _build_nc(mm_dt_name, BUFS=run_bufs) if run_bufs else _get_nc(
        mm_dt_name)
    in_maps = _prep_in_maps(x, Wq, bq, Wk, bk, Wv, bv, mm_dt_name)
    res = run_bass_kernel_spmd(nc, in_maps, core_ids=list(range(NCORES)),
                               **run_kw)
    scr = bool((run_bufs or {}).get("scr_out", 1))
    yl = []
    for c in range(NCORES):
        a = np.asarray(res.results[c]["y"]).astype(np.float32)
        if scr:
            a = a.reshape(NT, 16, 8, 8, D).transpose(0, 3, 1, 2, 4).reshape(
                TOK, F)
        yl.append(a)
    y = np.concatenate(yl, axis=0).reshape(B, N, F)
    return y, res


def kernel(x, Wq, bq, Wk, bk, Wv, bv):
    y, _ = run(x, Wq, bq, Wk, bk, Wv, bv, mm_dt_name="bf16")
    return y

